# revision 106
# baseline (speedup 1.0000x reference)
"""MixtureOfBlockAttention TRN2 kernel — 8-core head-parallel (TP) Bass/Tile implementation.

Semantics (verified equivalent to the reference, rel err ~2e-2 budget dominated
by top-k near-tie flips caused by fp32r rounding of x/w — irreducible without
dropping fp32r):
the reference mask `maximum(token_mask, causal*NEG_INF)` masks a position iff
it is BOTH future AND in a non-selected block. Consequences:
  - query blocks 0..7 attend to ALL tokens of key blocks 0..7 (dense, no mask);
  - query block i>=8 attends densely to key blocks 0..i-1, and within its own
    (diagonal) block applies strict causal masking ONLY for rows whose own
    block is not among their top-8 gating blocks.
Selection rank for query s in block i (i>=8): own block selected iff
  #{j < i : g[s,j] > g[s,i]} < 8, with g = q . (block sums of roped k)
(positive-scale invariant, so block sums replace means and the 1/sqrt(d)
factor is dropped).

Sharding: 16 query heads / 8 cores = 2 heads per core; KV head c serves both.
wq/wk/wv column-sliced, wo row-sliced; partial outputs summed on host.
Host-side layout prep: x is transposed to xT[c, s] (the PE contracts over the
partition dim, so both matmul operands need c on partitions) and float inputs
are pre-rounded to fp32r; both are pure data-layout transforms.

All big matmuls run in float32r (TF32-like input rounding, fp32 accumulate,
full PE rate at N>=256). Design (418.7us baseline -> 383.8us TimelineSim):
  - per-j exp fused across the 2 heads (one Act instruction, PSUM [128,2,512]);
  - softmax denominators WITHOUT per-j PE matmuls: each pexp is folded into
    one of two SBUF running-sum accumulators (tensor_tensor add on GPSIMD for
    odd j, DVE for even plus the chunk tail; the engines split the ~151K
    cycles the old ap=512 den matmuls burned on the PE). DVE adds are split
    per head so evacuation copies interleave between the halves on its
    in-order queue. The chunk's last pexp skips the accumulate and is
    contracted straight into the den bank, so the denominator closes one
    hop after the last exp. 6 boundary matmuls total;
  - flat cross-chunk software pipeline: QK/exp emission runs two global
    steps ahead of PV across chunk boundaries, so the next chunk's score/exp
    stream overlaps the previous chunk's normalize chain instead of queueing
    behind it in the in-order PE stream (boundaries used to cost 10-21us);
  - the normalize chain has NO PE ops except the den matmuls: reciprocal on
    DVE, then per-head partition_broadcast on GPSIMD (h1's row first moved
    to partition 0 by a tiny SWDGE SBUF->SBUF DMA: partition_broadcast
    requires a partition-0 source, HWDGE corrupts SBUF->SBUF partition
    moves, and StreamShuffle fails neuronxcc codegen);
  - band-mask tiles via GPSIMD partition_broadcast + DVE max (no PSUM, so
    the score slots stay free for the cross-boundary QK pipeline);
  - diagonal-band matmuls padded from ap=128 to ap=256 with a -1e5 PSUM
    memset in the pad so exp underflows to exact 0 there (fp32r runs
    4 cyc/row below ap 256);
  - wo projection emitted as per-chunk output-tile pieces fired one per
    attention step from a cross-chunk FIFO; PSUM->SBUF evacuation copies
    split 6/8 DVE : 2/8 Act; all out DMAs on the sync HWDGE queue (a DMA
    waiting on its source on the scalar queue head-of-line blocks the Act
    sequencer and stalls the exp stream, the attention pacer);
  - chunks 6-7's gating deferred into the attention phase as FIFO pieces
    (matmuls+compare chain, then per-head transpose batches);
  - phase-1 weight DMAs split small and q-pieces interleaved between k/v
    tails: the DMA wire serializes whole descriptors, and multi-tile weight
    blocks delayed the first x tiles (first matmul 9.7us -> ~5us);
  - rope tables by on-device angle-rotation recurrence; V-transpose and
    notflag-transpose in fp32r (1.5 cyc/row);
  - PSUM: scores 2x[128,2,512] + 2 ps_o + 1 ps_w + 1 den2 = exactly 8 banks.
"""

import math
import sys

import numpy as np

if "/opt/trn_rl_repo" not in sys.path:
    sys.path.insert(0, "/opt/trn_rl_repo")

import concourse.bacc as bacc
import concourse.mybir as mybir
import concourse.tile as tile
from concourse.bass_utils import run_bass_kernel_spmd

F32 = mybir.dt.float32
F32R = mybir.dt.float32r

SEQ = 4096
DIM = 2048
HEAD_DIM = 128
N_HEADS = 16
N_CORES = 8
HPC = N_HEADS // N_CORES       # heads per core = 2
DPC = HPC * HEAD_DIM           # q/o dims per core = 256
BLOCK = 128
NB = SEQ // BLOCK              # 32 key blocks
TOPK = 8
NCHUNK = 8                     # s-chunks of 512
CH = SEQ // NCHUNK             # 512
NCT = DIM // 128               # 16 contraction tiles
INV_SQRT_D = 1.0 / math.sqrt(HEAD_DIM)
PAD_NEG = -100000.0
_SWAP_MASK = [i ^ 1 for i in range(32)]

_CACHE = {}


def _round_fp32r(a):
    """Round fp32 to the fp32r grid (top-11-bit mantissa, round-to-nearest)."""
    a = np.ascontiguousarray(a, dtype=np.float32)
    try:
        from neuron_dtypes import static_cast_fp32_to_fp32r

        return static_cast_fp32_to_fp32r(a).view(np.float32).astype(np.float32)
    except Exception:
        u = a.view(np.uint32)
        return ((u + np.uint32(0x800)) & np.uint32(0xFFFFF000)).view(np.float32).copy()


def _host_constants():
    if "consts" in _CACHE:
        return _CACHE["consts"]
    p = np.arange(HEAD_DIM // 2, dtype=np.float64)
    inv_freq = 1.0 / (10000.0 ** (2.0 * p / HEAD_DIM))
    ang = np.arange(SEQ, dtype=np.float64)[None, :] * inv_freq[:, None]  # [64, S]
    cos = np.cos(ang).astype(np.float32)
    sin = np.sin(ang).astype(np.float32)
    cos_ds = np.ascontiguousarray(np.repeat(cos, 2, axis=0))   # [128, S]
    sin_ds = np.empty((HEAD_DIM, SEQ), dtype=np.float32)       # signed sin
    sin_ds[0::2] = -sin
    sin_ds[1::2] = sin
    # per-partition rotation by CH positions: next-chunk tables via
    # cos' = cos*C - sin_ds*S_row ; sin_ds' = sin_ds*C + cos*S_row
    # (S_row carries the sign convention of the interleaved sin_ds rows)
    inv_freq = 1.0 / (10000.0 ** (2.0 * p / HEAD_DIM))
    c512 = np.cos(CH * inv_freq)
    s512 = np.sin(CH * inv_freq)
    rotC = np.repeat(c512, 2).astype(np.float32)[:, None]      # [128, 1]
    rotS = np.empty((HEAD_DIM,), dtype=np.float64)
    rotS[0::2] = -s512
    rotS[1::2] = s512
    rotS = rotS.astype(np.float32)[:, None]                    # [128, 1]
    pswap = np.zeros((128, 128), dtype=np.float32)             # swap 2p <-> 2p+1
    idx = np.arange(128)
    pswap[idx, idx ^ 1] = 1.0
    identm = np.eye(128, dtype=np.float32)
    r = np.arange(BLOCK)
    trikeep = (r[:, None] <= r[None, :]).astype(np.float32)    # keep iff sk <= sq
    ones_row = np.ones((1, 128), dtype=np.float32)
    # one-hot-column stationaries for per-head den accumulation into [2, CH]:
    # oh2[:, h, :] is [128, 2] with column h all-ones
    oh2 = np.zeros((128, 2, 2), dtype=np.float32)
    oh2[:, 0, 0] = 1.0
    oh2[:, 1, 1] = 1.0
    # one-hot-row stationaries for per-head [2,CH] -> [128,CH] broadcast:
    # sel2[:, h, :] is [2, 128] with row h all-ones
    sel2 = np.zeros((2, 2, 128), dtype=np.float32)
    sel2[0, 0, :] = 1.0
    sel2[1, 1, :] = 1.0
    _CACHE["consts"] = (cos_ds, sin_ds, rotC, rotS, pswap, identm, trikeep, ones_row, oh2, sel2)
    return _CACHE["consts"]


def make_in_maps(x, wq, wk, wv, wo):
    """Shard + lay out the full inputs for the 8 cores."""
    x2 = np.asarray(x, dtype=np.float32).reshape(SEQ, DIM)
    xT = _round_fp32r(np.ascontiguousarray(x2.T))
    wq = np.asarray(wq, dtype=np.float32)
    wk = np.asarray(wk, dtype=np.float32)
    wv = np.asarray(wv, dtype=np.float32)
    wo = np.asarray(wo, dtype=np.float32)
    cos_ds, sin_ds, rotC, rotS, pswap, identm, trikeep, ones_row, oh2, sel2 = _host_constants()
    pswap_r = _round_fp32r(pswap)
    ones_row_r = _round_fp32r(ones_row)
    oh2_r = _round_fp32r(oh2)
    sel2_r = _round_fp32r(sel2)
    in_maps = []
    for c in range(N_CORES):
        in_maps.append(
            {
                "xT": xT,
                "wq": _round_fp32r(wq[:, c * DPC:(c + 1) * DPC]),
                "wk": _round_fp32r(wk[:, c * HEAD_DIM:(c + 1) * HEAD_DIM]),
                "wv": _round_fp32r(wv[:, c * HEAD_DIM:(c + 1) * HEAD_DIM]),
                "wo": _round_fp32r(wo[c * DPC:(c + 1) * DPC, :]),
                "cos0": np.ascontiguousarray(cos_ds[:, 0:CH]),
                "sin0": np.ascontiguousarray(sin_ds[:, 0:CH]),
                "rotC": rotC,
                "rotS": rotS,
                "pswap": pswap_r,
                "identm": _round_fp32r(identm),
                "trikeep": trikeep,
                "ones_r": ones_row_r,
                "oh2": oh2_r,
                "sel2": sel2_r,
            }
        )
    return in_maps


def _gating(nc, m, qT, bm, Ft, ident, ps_pool, ps_tag, sb_pool):
    """Own-block top-k flags for chunk m's 4 query blocks (both heads).

    All 8 gating matmuls first, then the DVE compare chains, then the 8
    transposes batched 4-per-PSUM-bank, so the in-order PE stream never
    waits mid-chain. PSUM scratch comes from (ps_pool, ps_tag) so this can
    run late, inside the attention phase, for the last two chunks.
    """
    import concourse.mybir as mybir

    F32 = mybir.dt.float32
    F32R = mybir.dt.float32r
    pairs = [(h, i) for h in range(HPC) for i in range(4 * m, 4 * m + 4)]
    nbk = 4 * m + 4  # even N; cols > i unused
    ps_g8 = ps_pool.tile([128, 8, NB], F32, tag=ps_tag, bufs=1, name="g8")
    for p, (h, i) in enumerate(pairs):
        nc.tensor.matmul(
            ps_g8[:, p, 0:nbk],
            qT[h][:, i * 128:(i + 1) * 128],
            bm[:, 0:nbk],
            start=True,
            stop=True,
        )
    nfs = []
    for p, (h, i) in enumerate(pairs):
        cmp = sb_pool.tile([128, NB], F32, tag="cmp", bufs=2, name="cmp")
        cnt = sb_pool.tile([128, 1], F32, tag="cnt", bufs=2, name="cnt")
        nc.vector.tensor_scalar(
            out=cmp[:, 0:i],
            in0=ps_g8[:, p, 0:i],
            scalar1=ps_g8[:, p, i:i + 1],
            scalar2=None,
            op0=mybir.AluOpType.is_gt,
        )
        nc.vector.tensor_reduce(
            cnt, cmp[:, 0:i], axis=mybir.AxisListType.X, op=mybir.AluOpType.add
        )
        # notflag: 1.0 -> own block selected (keep all)
        # (fp32r tiles: values are exactly 0.0/1.0)
        nf = sb_pool.tile([128, 1], F32R, tag="nf", bufs=8, name=f"nf{p}")
        nc.vector.tensor_scalar(
            out=nf,
            in0=cnt,
            scalar1=float(TOPK) - 0.5,
            scalar2=None,
            op0=mybir.AluOpType.is_lt,
        )
        nfs.append(nf)
    for h in range(HPC):
        ps_ft4 = ps_pool.tile([1, 4, 128], F32, tag=ps_tag, bufs=1, name="ft4")
        for t in range(4):
            nc.tensor.transpose(
                ps_ft4.bitcast(F32R)[:, t, :], nfs[4 * h + t], ident
            )
        nc.vector.tensor_copy(
            Ft[:, h, (4 * m - 8) * 128:(4 * m - 4) * 128],
            ps_ft4.rearrange("o f t -> o (f t)"),
        )



def _build_nc(reps=1):
    key = f"nc{reps}"
    if key in _CACHE:
        return _CACHE[key]
    nc = bacc.Bacc(None, target_bir_lowering=False)

    xT_d = nc.dram_tensor("xT", [DIM, SEQ], F32R, kind="ExternalInput")
    wq_d = nc.dram_tensor("wq", [DIM, DPC], F32R, kind="ExternalInput")
    wk_d = nc.dram_tensor("wk", [DIM, HEAD_DIM], F32R, kind="ExternalInput")
    wv_d = nc.dram_tensor("wv", [DIM, HEAD_DIM], F32R, kind="ExternalInput")
    wo_d = nc.dram_tensor("wo", [DPC, DIM], F32R, kind="ExternalInput")
    cos_d = nc.dram_tensor("cos0", [HEAD_DIM, CH], F32, kind="ExternalInput")
    sin_d = nc.dram_tensor("sin0", [HEAD_DIM, CH], F32, kind="ExternalInput")
    rotc_d = nc.dram_tensor("rotC", [HEAD_DIM, 1], F32, kind="ExternalInput")
    rots_d = nc.dram_tensor("rotS", [HEAD_DIM, 1], F32, kind="ExternalInput")
    psw_d = nc.dram_tensor("pswap", [128, 128], F32R, kind="ExternalInput")
    idm_d = nc.dram_tensor("identm", [128, 128], F32R, kind="ExternalInput")
    trk_d = nc.dram_tensor("trikeep", [BLOCK, BLOCK], F32, kind="ExternalInput")
    onr_d = nc.dram_tensor("ones_r", [1, 128], F32R, kind="ExternalInput")
    oh2_d = nc.dram_tensor("oh2", [128, 2, 2], F32R, kind="ExternalInput")
    sel2_d = nc.dram_tensor("sel2", [2, 2, 128], F32R, kind="ExternalInput")
    out_d = nc.dram_tensor("out", [SEQ, DIM], F32, kind="ExternalOutput")

    with tile.TileContext(nc) as tc, nc.allow_low_precision(
        reason="float32r rounding of matmul operands is intentional"
    ):
      for _rep in range(reps):
        with tc.tile_pool(name="persist", bufs=1) as per:
            qT = [per.tile([128, SEQ], F32R, tag=f"qT{h}", name=f"qT{h}") for h in range(HPC)]
            kT = per.tile([128, SEQ], F32R, tag="kT")
            vN = per.tile([128, NB, 128], F32R, tag="vN")   # [s-in-tile, sk-tile, d]
            ident = per.tile([128, 128], F32R, tag="ident")
            pswap = per.tile([128, 128], F32R, tag="pswap")
            trik = per.tile([BLOCK, BLOCK], F32, tag="trik")
            ones_r = per.tile([1, 128], F32R, tag="ones_r")
            oh2 = per.tile([128, 2, 2], F32R, tag="oh2")    # [k, h, den-col]
            sel2 = per.tile([2, 2, 128], F32R, tag="sel2")  # [den-row, h, p]
            bm = per.tile([128, NB], F32R, tag="bm")
            # per-head notflag rows: Ft[0, h, (i-TOPK)*128:...] is the [1,128]
            # notflag row for query block i of head h, at base partition 0
            Ft = per.tile([1, HPC, (NB - TOPK) * 128], F32R, tag="Ft")

            # dummy exp so the Exp act-table load overlaps the initial weight
            # DMAs instead of stalling the first attention chunk
            warm = per.tile([1, 1], F32, tag="warm")
            nc.vector.memset(warm, 0.0)
            nc.scalar.activation(
                out=warm, in_=warm, func=mybir.ActivationFunctionType.Exp
            )

            # ---------------- phase 1: projections + rope -------------------
            with (
                tc.tile_pool(name="wpool", bufs=1) as wp,
                tc.tile_pool(name="xtp", bufs=17) as xtp,
                tc.tile_pool(name="ropep", bufs=2) as rp,
                tc.tile_pool(name="csin", bufs=2) as csp,
                # acc_ps declared first so its PSUM range lines up with the
                # attention score pool's range: the last acc_ps readers (rope
                # copies) finish well before the gating tail that occupies
                # pj_ps, letting chunk-0 QK matmuls start during the tail
                tc.tile_pool(name="acc_ps", bufs=4, space="PSUM") as accps,
                tc.tile_pool(name="pj_ps", bufs=2, space="PSUM") as trps,
            ):
                wq_sb = wp.tile([128, NCT, DPC], F32R, tag="wq")
                wk_sb = wp.tile([128, NCT, HEAD_DIM], F32R, tag="wk")
                wv_sb = wp.tile([128, NCT, HEAD_DIM], F32R, tag="wv")
                wq_r = wq_d.rearrange("(t p) d -> p t d", p=128)
                wk_r = wk_d.rearrange("(t p) d -> p t d", p=128)
                wv_r = wv_d.rearrange("(t p) d -> p t d", p=128)
                # k/v weight heads ride the fast SP HWDGE queue (ahead of
                # the x tiles) so chunk 0's k/v matmuls start ~2us in; the
                # later-needed tails take the slow-dispatch SWDGE queue.
                # All transfers are split small: the DMA wire serializes
                # whole descriptors, so multi-tile weight blocks would delay
                # the first x tiles (and the first matmul) by several us.
                nc.sync.dma_start(out=wk_sb[:, 0:1, :], in_=wk_r[:, 0:1, :])
                nc.sync.dma_start(out=wv_sb[:, 0:1, :], in_=wv_r[:, 0:1, :])
                nc.sync.dma_start(out=wk_sb[:, 1:4, :], in_=wk_r[:, 1:4, :])
                nc.sync.dma_start(out=wv_sb[:, 1:4, :], in_=wv_r[:, 1:4, :])
                # interleave the q pieces between the k/v tails so the q
                # pass (~11us in) isn't stuck behind all six k/v transfers
                for i, t0 in enumerate(range(4, NCT, 4)):
                    nc.gpsimd.dma_start(
                        out=wk_sb[:, t0:t0 + 4, :], in_=wk_r[:, t0:t0 + 4, :]
                    )
                    nc.gpsimd.dma_start(
                        out=wv_sb[:, t0:t0 + 4, :], in_=wv_r[:, t0:t0 + 4, :]
                    )
                    q0 = 4 * i
                    nc.gpsimd.dma_start(
                        out=wq_sb[:, q0:q0 + 4, :], in_=wq_r[:, q0:q0 + 4, :]
                    )
                nc.gpsimd.dma_start(out=wq_sb[:, 12:16, :], in_=wq_r[:, 12:16, :])

                rot_c = wp.tile([128, 1], F32, tag="rotc")
                rot_s = wp.tile([128, 1], F32, tag="rots")
                gp = wp  # reuse the bufs=1 pool scope for small gating tiles
                for m in range(NCHUNK):
                    cols = slice(m * CH, (m + 1) * CH)
                    ps_q0 = accps.tile([128, CH], F32, tag="acc")
                    ps_q1 = accps.tile([128, CH], F32, tag="acc")
                    ps_k = accps.tile([128, CH], F32, tag="acc")
                    ps_v = accps.tile([128, CH], F32, tag="acc")
                    if m <= 1:
                        # two passes (k/v then q) to match the weight-arrival
                        # order; the xt tiles stay resident for the q pass
                        xts = []
                        for cc in range(NCT):
                            xt = xtp.tile([128, CH], F32R, tag="xt")
                            # first tiles ride the otherwise-idle Act HWDGE
                            # queue in parallel with the sync queue
                            q = nc.scalar if (m == 0 and cc < 4) else nc.sync
                            q.dma_start(
                                out=xt, in_=xT_d[cc * 128:(cc + 1) * 128, cols]
                            )
                            xts.append(xt)
                            st0, sp0 = (cc == 0), (cc == NCT - 1)
                            nc.tensor.matmul(ps_k, wk_sb[:, cc, :], xt, start=st0, stop=sp0)
                            nc.tensor.matmul(ps_v, wv_sb[:, cc, :], xt, start=st0, stop=sp0)

                        # consts (needed from the rope stage onwards) follow
                        # the early x tiles on the Act HWDGE queue
                        if m == 0:
                          # (rot tables first: needed by m=1's rope rotation;
                          # emitted after the early x tiles so they don't
                          # delay the first k/v matmuls on the scalar queue)
                          nc.scalar.dma_start(out=rot_c, in_=rotc_d[:])
                          nc.scalar.dma_start(out=rot_s, in_=rots_d[:])
                          nc.scalar.dma_start(out=pswap, in_=psw_d[:])
                          nc.scalar.dma_start(out=ident, in_=idm_d[:])
                          nc.scalar.dma_start(out=trik, in_=trk_d[:])
                          nc.scalar.dma_start(out=ones_r, in_=onr_d[:])
                          nc.scalar.dma_start(out=oh2, in_=oh2_d[:])
                          nc.scalar.dma_start(out=sel2, in_=sel2_d[:])
                        for cc in range(NCT):
                            st0, sp0 = (cc == 0), (cc == NCT - 1)
                            nc.tensor.matmul(ps_q0, wq_sb[:, cc, 0:128], xts[cc], start=st0, stop=sp0)
                            nc.tensor.matmul(ps_q1, wq_sb[:, cc, 128:256], xts[cc], start=st0, stop=sp0)
                    else:
                      for cc in range(NCT):
                        xt = xtp.tile([128, CH], F32R, tag="xt")
                        nc.sync.dma_start(
                            out=xt, in_=xT_d[cc * 128:(cc + 1) * 128, cols]
                        )
                        st0, sp0 = (cc == 0), (cc == NCT - 1)
                        nc.tensor.matmul(ps_q0, wq_sb[:, cc, 0:128], xt, start=st0, stop=sp0)
                        nc.tensor.matmul(ps_q1, wq_sb[:, cc, 128:256], xt, start=st0, stop=sp0)
                        nc.tensor.matmul(ps_k, wk_sb[:, cc, :], xt, start=st0, stop=sp0)
                        nc.tensor.matmul(ps_v, wv_sb[:, cc, :], xt, start=st0, stop=sp0)

                    if m == 0:
                        cos_t = csp.tile([128, CH], F32, tag="cos", bufs=2)
                        nc.scalar.dma_start(out=cos_t, in_=cos_d[:])
                        sin_t = csp.tile([128, CH], F32, tag="sin", bufs=2)
                        nc.scalar.dma_start(out=sin_t, in_=sin_d[:])
                    else:
                        # rotate the previous chunk's tables by CH positions
                        # (per-partition angle), off the DMA wire entirely
                        cos_p, sin_p = cos_t, sin_t
                        ta = rp.tile([128, CH], F32, tag="t2")
                        nc.vector.tensor_scalar(
                            out=ta, in0=sin_p, scalar1=rot_s, scalar2=None,
                            op0=mybir.AluOpType.mult,
                        )
                        cos_t = csp.tile([128, CH], F32, tag="cos", bufs=2)
                        nc.vector.scalar_tensor_tensor(
                            out=cos_t, in0=cos_p, scalar=rot_c, in1=ta,
                            op0=mybir.AluOpType.mult,
                            op1=mybir.AluOpType.subtract,
                        )
                        tb = rp.tile([128, CH], F32, tag="t2")
                        nc.vector.tensor_scalar(
                            out=tb, in0=cos_p, scalar1=rot_s, scalar2=None,
                            op0=mybir.AluOpType.mult,
                        )
                        sin_t = csp.tile([128, CH], F32, tag="sin", bufs=2)
                        nc.vector.scalar_tensor_tensor(
                            out=sin_t, in0=sin_p, scalar=rot_c, in1=tb,
                            op0=mybir.AluOpType.mult,
                            op1=mybir.AluOpType.add,
                        )

                    # V: evacuate then PE-transpose to natural [s, d] layout
                    # (fp32r copy: vN is fp32r anyway, and fp32r transpose runs
                    # 1.5 cyc/row vs 2.0 for fp32)
                    vtmp = rp.tile([128, CH], F32R, tag="qraw2")
                    nc.vector.tensor_copy(vtmp, ps_v)
                    ps_vt = trps.tile([128, CH], F32, tag="tr")
                    for u in range(4):
                        nc.tensor.transpose(
                            ps_vt.bitcast(F32R)[:, u * 128:(u + 1) * 128],
                            vtmp[:, u * 128:(u + 1) * 128],
                            ident,
                        )
                    nc.vector.tensor_copy(
                        vN[:, 4 * m:4 * m + 4, :],
                        ps_vt.rearrange("p (u d) -> p u d", u=4),
                    )

                    # all three raw copies first: they are the last readers
                    # of the projection PSUM banks, and freeing those banks
                    # early unblocks the next chunk's matmuls (and, for the
                    # last chunk, the attention start) via the address WAR
                    raws = []
                    for psrc in (ps_q0, ps_q1, ps_k):
                        raw = rp.tile([128, CH], F32R, tag="qraw", bufs=3)
                        nc.vector.tensor_copy(raw, psrc)
                        raws.append(raw)
                    for raw, dstT in zip(raws, (qT[0], qT[1], kT)):
                        ps_sw = trps.tile([128, CH], F32, tag="tr")
                        nc.tensor.matmul(ps_sw, pswap, raw, start=True, stop=True)
                        t2 = rp.tile([128, CH], F32, tag="t2")
                        nc.vector.tensor_tensor(
                            t2, raw.bitcast(F32), cos_t, op=mybir.AluOpType.mult
                        )
                        # sw *= sin in place (PSUM), then add -> rope output
                        nc.vector.tensor_tensor(ps_sw, ps_sw, sin_t, op=mybir.AluOpType.mult)
                        nc.vector.tensor_tensor(
                            dstT[:, cols], t2, ps_sw, op=mybir.AluOpType.add
                        )

                    # partial block sums for this chunk's 4 key blocks
                    nc.vector.tensor_reduce(
                        bm[:, 4 * m:4 * m + 4],
                        kT.bitcast(F32)[:, cols].rearrange("p (b t) -> p b t", b=4),
                        axis=mybir.AxisListType.X,
                        op=mybir.AluOpType.add,
                    )
                    # gating flags for this chunk's query blocks (needs
                    # bm 0..i); chunks 6-7 are deferred into the attention
                    # phase so the phase boundary is not serialized on them
                    if 2 <= m <= 5:
                        _gating(nc, m, qT, bm, Ft, ident, trps, "g", gp)

            # ---------------- phases 3+4 ------------------------------------
            _phase34(nc, tc, qT, kT, vN, trik, oh2, sel2, ones_r, Ft, wo_d,
                     out_d, bm, ident)

    nc.compile()
    _CACHE[key] = nc
    return nc


def _phase34(nc, tc, qT, kT, vN, trik, oh2, sel2, ones_r, Ft, wo_d, out_d,
             bm, ident):
    wop_cm = tc.tile_pool(name="wop", bufs=1)
    wop = wop_cm.__enter__()
    wo_sb = wop.tile([128, HPC, DIM], F32R, tag="wo")
    nc.sync.dma_start(out=wo_sb, in_=wo_d.rearrange("(t p) d -> p t d", p=128))
    # ------- phase 3: attention with interleaved output projection -------
    # (wo(m) right after attn(m) so the 32MB output DMA spreads over the
    # whole kernel instead of piling into a DMA-bound tail phase)
    # PSUM budget (16KB/partition): pss "s" 2x[128,2,CH] = 8KB, pso "o"
    # 2x[128,CH] + "w" 1x[128,CH] = 6KB, psd "den2" 1x[2,CH] = 2KB.
    # Broadcast scratch and the final wo emission reuse the pss "s" slots.
    # ps_w gets its own tag so the deferred wo matmuls interleave freely into
    # the attention exp-latency gaps instead of queueing behind ps_o's WAR.
    with (
        tc.tile_pool(name="att", bufs=4) as ap,
        tc.tile_pool(name="attb", bufs=2) as ab,
        tc.tile_pool(name="oTs", bufs=4) as otp,
        tc.tile_pool(name="outp", bufs=8) as outp,
        tc.tile_pool(name="sacc", bufs=2) as sap,
        tc.tile_pool(name="bcast", bufs=2) as bcp,
        tc.tile_pool(name="att_s", bufs=2, space="PSUM") as pss,
        tc.tile_pool(name="att_o", bufs=2, space="PSUM") as pso,
        tc.tile_pool(name="att_d", bufs=1, space="PSUM") as psd,
    ):
        # ---- wo(m') emission: projection for s-tiles of chunk m' ----
        # Emitted piecewise, one output tile per attention j-iteration of the
        # NEXT chunk, so the wo matmuls fill the PE's exp-latency gaps.
        def wo_pieces(mm, oTc_mm, final=False):
            for u in range(4):
                st = 4 * mm + u
                osb2 = None
                for n in range(4):
                    ncols = slice(n * 512, (n + 1) * 512)
                    if final or piece_slot[0] == "f":
                        # score slots are idle at kernel end: a 2-deep
                        # pipelined drain instead of the single-bank "w"
                        # WAR chain
                        ps_w2 = pss.tile([128, HPC, CH], F32, tag="s", name="psw2")
                        ps_w = ps_w2[:, 0, :]
                    else:
                        ps_w = pso.tile([128, 512], F32, tag="w", name="psw", bufs=1)
                    nc.tensor.matmul(
                        ps_w,
                        oTc_mm[0][:, u * 128:(u + 1) * 128],
                        wo_sb[:, 0, ncols],
                        start=True,
                        stop=False,
                    )
                    nc.tensor.matmul(
                        ps_w,
                        oTc_mm[1][:, u * 128:(u + 1) * 128],
                        wo_sb[:, 1, ncols],
                        start=False,
                        stop=True,
                    )
                    osb = outp.tile([128, 512], F32, tag="ow")
                    # PSUM->SBUF copies can only run on DVE or Act (GPSIMD
                    # has no PSUM port). DVE carries the den adds now, so Act
                    # takes 3/8 of the copies steady-state and half in the
                    # final drain (where Act is idle).
                    idx = st * 4 + n
                    act_copy = (
                        idx % 2 == 0 if piece_slot[0] == "f" else idx % 4 == 1
                    )
                    if act_copy:
                        nc.scalar.copy(osb, ps_w)
                    else:
                        nc.vector.tensor_copy(osb, ps_w)
                    # all out tiles ride the sync HWDGE queue: its SP
                    # sequencer has no compute engine behind it, so a DMA
                    # waiting on its source can't head-of-line block the exp
                    # stream (scalar queue) or the den adds (SWDGE/Pool)
                    nc.sync.dma_start(
                        out=out_d[st * 128:(st + 1) * 128, ncols], in_=osb
                    )
                    yield True

        wo_queue = []  # pending wo piece generators (FIFO across chunks)
        piece_slot = ["w"]  # PSUM scratch for the next piece ("w" or "s")

        def fire_wo(n=1):
            # at most n pieces; a second piece per attention step would stall
            # the in-order PE stream on the single-bank ps_w WAR
            while n > 0 and wo_queue:
                if next(wo_queue[0], None) is None:
                    wo_queue.pop(0)
                else:
                    n -= 1

        # ---- flat cross-chunk software pipeline over all (m, j) steps ----
        # QK/exp emission runs two global steps ahead of PV emission and
        # crosses chunk boundaries, so the next chunk's score matmuls and
        # exps overlap the previous chunk's normalize chain instead of
        # queueing behind it in the in-order PE stream. The normalize chain
        # keeps only the 4 den matmuls on the PE; the reciprocal broadcast
        # and the band-mask broadcast run on GPSIMD (partition_broadcast),
        # which is idle at boundaries.
        steps = []
        for m in range(NCHUNK):
            nsk = 8 if m < 2 else 4 * m + 4
            steps.extend((m, j, nsk) for j in range(nsk))

        state = {}

        def ensure_chunk(m):
            if m in state:
                return
            state[m] = {
                "nsk": 8 if m < 2 else 4 * m + 4,
                "ps_o": [
                    pso.tile([128, CH], F32, tag="o", name=f"o{h}")
                    for h in range(HPC)
                ],
                "oTc": [
                    otp.tile([128, CH], F32R, tag="oTc", name=f"oTc{h}")
                    for h in range(HPC)
                ],
                # running exp-sum accumulators for the softmax denominators:
                # the per-j ap=512 den matmuls are off the PE; each pexp is
                # folded into one of two SBUF accumulators (GPSIMD for even
                # j, DVE for odd), contracted by 4 matmuls at chunk end
                "S_acc": [
                    sap.tile([128, HPC, CH], F32R, tag=t, name=t)
                    for t in ("sd", "sp")
                ],
                "started": [False, False],
                "ps_den": psd.tile([2, CH], F32, tag="den2", name="den2", bufs=1),
                "mks": None,
            }

        def colspan(m, j):
            band = m >= 2 and j >= 4 * m
            # pad ap=128 matmuls (4 cyc/row below ap 256) to ap=256
            col0 = (j - 4 * m) * 128 if band else 0
            colp = min(col0, CH - 256) if band else 0
            return band, col0, colp

        def emit_qk_exp(m, j):
            st = state[m]
            band, col0, colp = colspan(m, j)
            ps_s = pss.tile([128, HPC, CH], F32, tag="s", bufs=2)
            for h in range(HPC):
                nc.tensor.matmul(
                    ps_s[:, h, colp:],
                    kT[:, j * 128:(j + 1) * 128],
                    qT[h][:, m * CH + colp:(m + 1) * CH],
                    start=True,
                    stop=True,
                )
            if colp < col0:
                # overwrite the pad region so exp underflows to exact 0
                nc.vector.memset(ps_s[:, :, colp:col0], PAD_NEG)
            pexp = ap.tile([128, HPC, CH], F32R, tag="pexp", bufs=7)
            nc.scalar.activation(
                out=pexp[:, :, colp:],
                in_=ps_s[:, :, colp:],
                func=mybir.ActivationFunctionType.Exp,
                scale=INV_SQRT_D,
            )
            if band:
                nc.vector.tensor_tensor(
                    pexp[:, :, col0:col0 + 128],
                    pexp.bitcast(F32)[:, :, col0:col0 + 128],
                    st["mks"][:, :, col0:col0 + 128],
                    op=mybir.AluOpType.mult,
                )
            # exp-sum accumulate (after the band mask mult). The pad region
            # [colp:col0) holds exp(scale*PAD_NEG) == 0 exactly, and columns
            # < colp get no contribution from key block j. The chunk's LAST
            # pexp skips the SBUF accumulate: it is contracted directly into
            # ps_den by the boundary matmuls, so the denominator is ready
            # one engine-hop after the last exp instead of waiting a full
            # DVE/GPSIMD accumulate round trip.
            if j == st["nsk"] - 1:
                st["pexp_last"] = pexp
                return pexp
            # evens go to GPSIMD except the tail: the last adds before the
            # boundary stay on the faster DVE so the den matmuls aren't
            # gated on the slow GPSIMD queue
            a = 0 if (j % 2 == 1 and j < st["nsk"] - 3) else 1
            acc = st["S_acc"][a]
            if not st["started"][a]:
                eng = nc.gpsimd if a == 0 else nc.vector
                eng.tensor_copy(acc[:, :, colp:], pexp[:, :, colp:])
                st["started"][a] = True
            elif a == 0:
                nc.gpsimd.tensor_tensor(
                    acc[:, :, colp:],
                    acc[:, :, colp:],
                    pexp[:, :, colp:],
                    op=mybir.AluOpType.add,
                )
            else:
                # per-head halves: finer DVE queue granularity lets the
                # piece-evacuation copies interleave between the halves
                # instead of waiting out a full-width add
                for h in range(HPC):
                    nc.vector.tensor_tensor(
                        acc[:, h, colp:],
                        acc[:, h, colp:],
                        pexp[:, h, colp:],
                        op=mybir.AluOpType.add,
                    )
            return pexp

        def emit_pv(m, j, pexp):
            st = state[m]
            _, _, colp = colspan(m, j)
            for h in range(HPC):
                nc.tensor.matmul(
                    st["ps_o"][h][:, colp:],
                    vN[:, j, :],
                    pexp[:, h, colp:],
                    start=(j == 0),
                    stop=(j == st["nsk"] - 1),
                )

        def emit_mks(m):
            # band-mask tiles for chunk m: broadcast the per-head notflag
            # rows over partitions on GPSIMD, then max with the triangular
            # keep mask on DVE. No PSUM banks or PE matmuls involved, so
            # the score slots stay free for the cross-boundary QK pipeline.
            st = state[m]
            bcF = bcp.tile([128, HPC, CH], F32R, tag="bcF", bufs=2, name="bcF")
            for h in range(HPC):
                nc.gpsimd.partition_broadcast(
                    bcF[:, h, :],
                    Ft[:, h, (4 * m - 8) * 128:(4 * m - 4) * 128],
                    channels=128,
                )
            mks = ab.tile([128, HPC, CH], F32, tag="mk", bufs=2)
            trik_b = trik.rearrange("p (a b t) -> p a b t", a=1, b=1).broadcast_to(
                [128, HPC, 4, BLOCK]
            )
            nc.vector.tensor_tensor(
                mks.rearrange("p h (b t) -> p h b t", b=4),
                trik_b,
                bcF.bitcast(F32).rearrange("p h (b t) -> p h b t", b=4),
                op=mybir.AluOpType.max,
            )
            st["mks"] = mks

        def gating_pieces(gm):
            # deferred top-k gating for blocks 4gm..4gm+3 (same math as
            # _gating), split into FIFO pieces so the DVE compare chain and
            # the Ft transposes interleave with attention steps instead of
            # serializing a chunk boundary
            pairs = [(h, i) for h in range(HPC) for i in range(4 * gm, 4 * gm + 4)]
            nbk = 4 * gm + 4
            ps_g8 = pso.tile([128, 8, NB], F32, tag="w", bufs=1, name="g8")
            for p, (h, i) in enumerate(pairs):
                nc.tensor.matmul(
                    ps_g8[:, p, 0:nbk],
                    qT[h][:, i * 128:(i + 1) * 128],
                    bm[:, 0:nbk],
                    start=True,
                    stop=True,
                )
            yield True
            nfs = []
            for p, (h, i) in enumerate(pairs):
                cmp = ab.tile([128, NB], F32, tag="cmp", bufs=2, name="cmp")
                cnt = ab.tile([128, 1], F32, tag="cnt", bufs=2, name="cnt")
                nc.vector.tensor_scalar(
                    out=cmp[:, 0:i],
                    in0=ps_g8[:, p, 0:i],
                    scalar1=ps_g8[:, p, i:i + 1],
                    scalar2=None,
                    op0=mybir.AluOpType.is_gt,
                )
                nc.vector.tensor_reduce(
                    cnt, cmp[:, 0:i], axis=mybir.AxisListType.X,
                    op=mybir.AluOpType.add,
                )
                nf = ab.tile([128, 1], F32R, tag="nf", bufs=8, name=f"nf{p}")
                nc.vector.tensor_scalar(
                    out=nf,
                    in0=cnt,
                    scalar1=float(TOPK) - 0.5,
                    scalar2=None,
                    op0=mybir.AluOpType.is_lt,
                )
                nfs.append(nf)
                # fine-grained yields keep the DVE compare chain from
                # clogging the queue in one burst (it delayed the osb copies
                # and stalled the PE on the w-bank WAR at boundaries)
                if p % 2 == 1:
                    yield True
            for h in range(HPC):
                ps_ft4 = pso.tile([1, 4, 128], F32, tag="w", bufs=1, name="ft4")
                for t in range(4):
                    nc.tensor.transpose(
                        ps_ft4.bitcast(F32R)[:, t, :], nfs[4 * h + t], ident
                    )
                nc.vector.tensor_copy(
                    Ft[:, h, (4 * gm - 8) * 128:(4 * gm - 4) * 128],
                    ps_ft4.rearrange("o f t -> o (f t)"),
                )
                yield True

        def boundary(m):
            st = state[m]
            # contract the two exp-sum accumulators plus the last step's raw
            # pexp into the denominator PSUM rows: oh2[:, h, :] routes head
            # h's sum to partition h
            _, _, colp_l = colspan(m, st["nsk"] - 1)
            for h in range(HPC):
                nc.tensor.matmul(
                    st["ps_den"],
                    oh2[:, h, :],
                    st["S_acc"][1][:, h, :],
                    start=(h == 0),
                    stop=False,
                )
            for h in range(HPC):
                nc.tensor.matmul(
                    st["ps_den"][:, colp_l:],
                    oh2[:, h, :],
                    st["pexp_last"][:, h, colp_l:],
                    start=False,
                    stop=False,
                )
            for h in range(HPC):
                nc.tensor.matmul(
                    st["ps_den"],
                    oh2[:, h, :],
                    st["S_acc"][0][:, h, :],
                    start=False,
                    stop=(h == HPC - 1),
                )
            st["pexp_last"] = None
            # reciprocal, then broadcast each head's row over partitions on
            # GPSIMD. partition_broadcast requires a partition-0 source, so
            # head 1's row is first moved to partition 0 of a scratch tile
            # by a tiny sync-queue SBUF->SBUF DMA (2KB): the whole normalize
            # chain stays off the in-order PE stream.
            rec2 = ab.tile([2, CH], F32R, tag="rec")
            nc.vector.reciprocal(rec2, st["ps_den"])
            bc = bcp.tile([128, CH], F32R, tag="rbc", bufs=2, name="rbc0")
            nc.gpsimd.partition_broadcast(bc, rec2[0:1, :], channels=128)
            nc.vector.tensor_tensor(
                st["oTc"][0], st["ps_o"][0], bc.bitcast(F32),
                op=mybir.AluOpType.mult,
            )
            if m == 2:
                wo_queue.append(gating_pieces(6))
            elif m == 3:
                wo_queue.append(gating_pieces(7))
            wo_queue.append(wo_pieces(m, st["oTc"], final=(m == NCHUNK - 1)))
            if 2 <= m + 1 < NCHUNK:
                mks_pending.append(m + 1)
            # head 1's reciprocal row must reach partition 0 for the GPSIMD
            # broadcast; only a SWDGE SBUF->SBUF DMA moves partitions
            # correctly (the HWDGE queues corrupt it). Emitted last so the
            # trigger's wait on the reciprocal can't head-of-line block the
            # independent Pool work above.
            rec_sw = ab.tile([1, CH], F32R, tag="recsw", bufs=2, name="rec_sw")
            nc.gpsimd.dma_start(out=rec_sw, in_=rec2[1:2, :])
            bc1 = bcp.tile([128, CH], F32R, tag="rbc", bufs=2, name="rbc1")
            nc.gpsimd.partition_broadcast(bc1, rec_sw[0:1, :], channels=128)
            nc.vector.tensor_tensor(
                st["oTc"][1], st["ps_o"][1], bc1.bitcast(F32),
                op=mybir.AluOpType.mult,
            )

        mks_pending = []
        pexps = {}
        ensure_chunk(0)
        pexps[0] = emit_qk_exp(steps[0][0], steps[0][1])
        pexps[1] = emit_qk_exp(steps[1][0], steps[1][1])
        for idx, (m, j, nsk) in enumerate(steps):
            la = idx + 2
            if la < len(steps):
                m2, j2, _ = steps[la]
                ensure_chunk(m2)
                pexps[la] = emit_qk_exp(m2, j2)
            if mks_pending:
                emit_mks(mks_pending.pop(0))
            fire_wo(1)
            emit_pv(m, j, pexps.pop(idx))
            if j == nsk - 1:
                boundary(m)
        piece_slot[0] = "f"
        while wo_queue:
            fire_wo(1)
    wop_cm.__exit__(None, None, None)


def kernel(x, wq, wk, wv, wo):
    bs = np.asarray(x).shape[0]
    in_maps = make_in_maps(x, wq, wk, wv, wo)
    nc = _build_nc()
    res = run_bass_kernel_spmd(nc, in_maps, list(range(N_CORES)))
    out = res.results[0]["out"].astype(np.float64)
    for c in range(1, N_CORES):
        out += res.results[c]["out"]
    return out.astype(np.float32).reshape(bs, SEQ, DIM)


if __name__ == "__main__":
    rng = np.random.default_rng(0)
    xs = {
        "x": rng.standard_normal((1, SEQ, DIM), dtype=np.float32),
        "wq": rng.standard_normal((DIM, DIM), dtype=np.float32) * (DIM ** -0.5),
        "wk": rng.standard_normal((DIM, DIM // 2), dtype=np.float32) * (DIM ** -0.5),
        "wv": rng.standard_normal((DIM, DIM // 2), dtype=np.float32) * (DIM ** -0.5),
        "wo": rng.standard_normal((DIM, DIM), dtype=np.float32) * (DIM ** -0.5),
    }
    out = kernel(**xs)
    print("out", out.shape, out.dtype, np.abs(out).max())

